# revision 1
# baseline (speedup 1.0000x reference)
"""Butterfly rotation (10 stages, DIM=1024) on 8 Trainium2 NeuronCores.

Math: the 10-stage butterfly transform is linear.  Stages 0..8 (strides
1..256) mix only within 512-wide blocks: their composite is block-diagonal
with two dense 512x512 blocks, applied on the PE as fp16 matmuls (the
2e-2 error budget admits fp16 end-to-end; measured pipeline error ~1e-3).
Stage 9 (stride 512) pairs chunk cg with cg+4 (128-dim chunks) and is an
elementwise per-partition rotation.

Measured per-[128,1024]-pass engine menu (this deployment): ACT copy from
PSUM 731 ns / with [128,1] scale 1025 ns; DVE fp16 stt 815 ns / TT 635 ns
(SBUF; PSUM-sourced DVE is 1.8 us - avoided); GPSIMD ~8-15 us (unusable);
PE fp16 [128,128,512] matmul 98 ns; DMA floor ~41 us.  PSUM eviction must
therefore ride ACT, stage-9 combines ride DVE in SBUF, and the PE (which
has slack under the DMA roofline) absorbs some chunk pairs entirely as
dense 1024-wide rows of the full 10-stage composite.

Per stage-9 pair (cg, cg+4), three implementations (mix is tuned to
balance engines):
  dense:  PE computes the final outputs via dense rows (16 MM/chunk);
          ACT evicts straight into the output tile.  No DVE work.
  shear:  PE computes stage-0..8 chunks z (8 MM/chunk, with the stage-9
          sign sg = sign(cos th9) folded into the weight rows so the
          residual rotation angle th9' has cos >= 0); ACT evicts z to
          fp16; DVE applies the rotation as 3 shears
          (u += a.v; v += s.u; u += a.v with a = -tan(th9'/2), |a| <= 1).
  acth:   like shear but ACT evicts 4 scaled copies (c9'.z_lo, -s9'.z_hi,
          s9'.z_lo, c9'.z_hi) and DVE just adds pairs (2 TT) - shifts
          work from DVE to ACT.

Device layout (per core, rows sharded 8192/core; pure data parallelism):
  host packs each core's shard dim-major fp16:
  xin[g, p, c*1024 + r] = x[g*1024 + r, c*128 + p]  (g: 8 row-groups of
  1024 rows, c: 8 dim-chunks of 128, p: dim-within-chunk).  Loads ride
  the SP HWDGE ring, stores the ACT ring, ~1 MiB per transfer.  Host
  inverse-permutes and upcasts the fp16 output.
"""

import os
import sys

sys.path.insert(0, "/opt/trn_rl_repo")

# run_bass_kernel_spmd would try to import the (absent) axon NTFF hook if
# BASS_TRACE is set in the environment.
os.environ["BASS_NEVER_TRACE"] = "1"

import numpy as np

DIM = 1024
STAGES = 10
N_CORES = 8
ROWS_PER_CORE = 8192
GROUP_ROWS = 1024
N_GROUPS = ROWS_PER_CORE // GROUP_ROWS  # 8

# stage-9 pair cg -> implementation; tuned so PE/ACT/DVE land together
PAIR_MODES = ["dense", "shear", "shear", "shear"]


def _stage_idx(dim, stage):
    stride = 2**stage
    idx_i = np.arange(dim).reshape(-1, 2 * stride)[:, :stride].ravel()
    idx_j = idx_i + stride
    return idx_i, idx_j


def _butterfly_apply(v, angles, stages):
    """Apply butterfly stages to rows of v (float64, in place) and return v."""
    for s in stages:
        idx_i, idx_j = _stage_idx(v.shape[1], s)
        c = np.cos(angles[s].astype(np.float64))
        sn = np.sin(angles[s].astype(np.float64))
        vi = v[:, idx_i].copy()
        vj = v[:, idx_j].copy()
        v[:, idx_i] = c * vi - sn * vj
        v[:, idx_j] = sn * vi + c * vj
    return v


def _host_tables(angles):
    """w9[k, c*4+t, m] fp16: lhsT for stage-0..8 output chunk c from input
    chunk ci = 4*(c//4) + t, rows pre-scaled by sg9[c % 4] (stage-9 sign).
    wd[k, c*8+t, m] fp16: dense full-composite lhsT, output chunk c from
    input chunk t.
    trig[p, 0..3]=a9[cg], [4..7]=sin th9', [8..11]=cos th9', [12..15]=-sin.
    """
    # _butterfly_apply on eye gives mb[i, j] = M[j, i] (M maps in->out),
    # so lhsT[k, m] = M[c*128+m, ci*128+k] = mb[ci*128+k, c*128+m].
    mb9 = _butterfly_apply(np.eye(DIM, dtype=np.float64), angles, range(9))
    mbA = _butterfly_apply(mb9.copy(), angles, [9])
    mask = np.ones((DIM, DIM), dtype=bool)
    for h in range(2):
        mask[h * 512 : (h + 1) * 512, h * 512 : (h + 1) * 512] = False
    assert abs(mb9[mask]).max() == 0.0

    th9 = angles[9].astype(np.float64)
    sg9, cth, sth, a9 = [], [], [], []
    for cg in range(4):
        th = th9[cg * 128 : (cg + 1) * 128]
        sg = np.where(np.cos(th) >= 0, 1.0, -1.0)
        c9, s9 = sg * np.cos(th), sg * np.sin(th)
        sg9.append(sg)
        cth.append(c9)
        sth.append(s9)
        a9.append(-np.tan(np.arctan2(s9, c9) / 2))
        assert np.abs(a9[-1]).max() <= 1.0 + 1e-12

    w9 = np.empty((128, 32, 128), dtype=np.float16)
    for c in range(8):
        for t in range(4):
            ci = 4 * (c // 4) + t
            blk = mb9[ci * 128 : (ci + 1) * 128, c * 128 : (c + 1) * 128]
            w9[:, c * 4 + t, :] = (blk * sg9[c % 4][None, :]).astype(np.float16)
    wd = np.empty((128, 64, 128), dtype=np.float16)
    for c in range(8):
        for t in range(8):
            wd[:, c * 8 + t, :] = mbA[
                t * 128 : (t + 1) * 128, c * 128 : (c + 1) * 128
            ].astype(np.float16)

    trig = np.empty((128, 16), dtype=np.float32)
    for cg in range(4):
        trig[:, 0 + cg] = a9[cg]
        trig[:, 4 + cg] = sth[cg]
        trig[:, 8 + cg] = cth[cg]
        trig[:, 12 + cg] = -sth[cg]
    return w9, wd, trig


def _pack_x(x_core, n_groups=N_GROUPS):
    # [G*1024, 1024] -> [G, 128, 8192] with xin[g, p, c*1024+r] = x[g*1024+r, c*128+p]
    g = x_core.reshape(n_groups, GROUP_ROWS, 8, 128)
    return np.ascontiguousarray(
        g.transpose(0, 3, 2, 1).reshape(n_groups, 128, 8 * GROUP_ROWS)
    )


# device output slot s holds chunk SLOT_CHUNK[s]: pair (cg, cg+4) lands in
# adjacent slots (2cg, 2cg+1) so each pair's PSUM/eviction is contiguous
SLOT_CHUNK = [0, 4, 1, 5, 2, 6, 3, 7]


def _unpack_y(y_packed, n_groups=N_GROUPS):
    # yout[g, p, slot(c)*1024 + r] = y[g*1024 + r, c*128 + p]
    g = y_packed.reshape(n_groups, 128, 8, GROUP_ROWS)
    g = g.transpose(0, 3, 2, 1)  # [g, r, slot, p]
    chunk_slot = np.argsort(SLOT_CHUNK)  # chunk -> slot
    g = g[:, :, chunk_slot, :]
    return np.ascontiguousarray(g.reshape(n_groups * GROUP_ROWS, DIM))


def _patch_tile_drain():
    """Workaround: this walrus build cannot encode semaphore waits on a
    sequencer Drain/NoOp with >1 wait ("Too many sync wait commands").
    Re-emit the TileContext tail waits as one nop per semaphore."""
    from concourse import mybir, tile
    from concourse.vector_clock import ScopedClock

    if getattr(tile.TileContext, "_drain_patched", False):
        return

    def _drain_and_barrier(self, tick_clock, wait_clock):
        nop_inst = self.nc.sync.nop(nofuse=True)
        wait_clock.add_sem_waits(
            nop_inst.ins, ScopedClock({None: tick_clock.global_clock})
        )
        si = nop_inst.ins.sync_info
        if si is not None and si.on_wait and len(si.on_wait) > 1:
            extra = si.on_wait[1:]
            si.on_wait = si.on_wait[:1]
            for w in extra:
                extra_nop = self.nc.sync.nop(nofuse=True)
                esi = extra_nop.ins.sync_info
                if esi is None:
                    extra_nop.ins.sync_info = mybir.SyncInfo(on_wait=[w], on_update=[])
                else:
                    esi.on_wait = list(esi.on_wait or []) + [w]
        self.nc.sync.drain()
        self.nc.all_engine_barrier()
        assert self.sems is not None
        popped = self.nc._tile_sem_poison_stack.pop()
        assert popped is self._sem_poison
        self.nc.clear_and_free_semaphores(list(self.sems.allocated().values()))
        self.nc.all_engine_barrier()

    tile.TileContext._drain_and_barrier = _drain_and_barrier
    tile.TileContext._drain_patched = True


def _split_multi_waits(nc, limit=1):
    """This walrus build encodes at most `limit` semaphore wait(s) per
    instruction ("Too many sync wait commands").  Hoist excess waits onto
    same-engine NoOps inserted immediately before the instruction."""
    from concourse import mybir

    counter = [0]

    def fresh_nop(engine, waits):
        counter[0] += 1
        nop = mybir.InstNoOp(
            name=f"waitsplit-{counter[0]}",
            engine=engine,
            ins=[],
            outs=[],
            bass_nofuse=True,
            sync_info=mybir.SyncInfo(on_wait=list(waits), on_update=[]),
        )
        nc.register_instruction(nop, overwrite=True)
        return nop

    for fn in nc.m.functions:
        for bb in fn.blocks:
            changed = False
            new = []
            for inst in bb.instructions:
                si = getattr(inst, "sync_info", None)
                if si is not None and si.on_wait and len(si.on_wait) > limit:
                    extra = si.on_wait[: len(si.on_wait) - limit]
                    si.on_wait = si.on_wait[len(si.on_wait) - limit :]
                    for k in range(0, len(extra), limit):
                        new.append(fresh_nop(inst.engine, extra[k : k + limit]))
                    changed = True
                new.append(inst)
            if changed:
                bb.instructions = new
    return nc


def build_bass(n_groups=N_GROUPS, reps=1, pair_modes=None, upto="full"):
    """Build the Bass module for one core processing n_groups row-groups.
    reps>1 repeats the whole pipeline in-NEFF (for timing calibration).
    upto: 'pe' | 'evict' | 'dve' | 'full' truncates the pipeline (for
    engine-attribution benchmarks)."""
    _patch_tile_drain()
    from concourse import bass, mybir, tile

    static_x = upto in ("penoload", "pedummyload")
    dummy_load = upto == "pedummyload"
    if static_x:
        upto = "pe"
    stage_n = ["pe", "evict", "dve", "full"].index(upto)
    pair_modes = pair_modes or PAIR_MODES
    f16 = mybir.dt.float16
    f32 = mybir.dt.float32
    nc = bass.Bass("TRN2", target_bir_lowering=False, debug=False)
    xin = nc.dram_tensor("xin", [n_groups, 128, 8192], f16, kind="ExternalInput")
    w9d = nc.dram_tensor("w9", [128, 32, 128], f16, kind="ExternalInput")
    wdd = nc.dram_tensor("wd", [128, 64, 128], f16, kind="ExternalInput")
    trig = nc.dram_tensor("trig", [128, 16], f32, kind="ExternalInput")
    yout = nc.dram_tensor("yout", [n_groups, 128, 8192], f16, kind="ExternalOutput")

    mult = mybir.AluOpType.mult
    add = mybir.AluOpType.add
    copy_fn = mybir.ActivationFunctionType.Copy

    def mm_pair(psum, wtile, cg, nw, xt):
        """Fill a pair's [128, 2048] PSUM tile: chunk cg in cols 0:1024,
        chunk cg+4 in 1024:2048, each accumulated from nw input chunks."""
        for side, c in enumerate((cg, cg + 4)):
            for t in range(nw):
                ci = (c // 4) * 4 + t if nw == 4 else t
                for h in range(2):
                    nc.tensor.matmul(
                        psum[:, side * 1024 + h * 512 : side * 1024 + (h + 1) * 512],
                        wtile[:, c * nw + t, :],
                        xt[:, ci * 1024 + h * 512 : ci * 1024 + (h + 1) * 512],
                        start=(t == 0),
                        stop=(t == nw - 1),
                    )

    with tile.TileContext(nc) as tc:
        with (
            tc.tile_pool(name="wp", bufs=1) as wp,
            tc.tile_pool(name="xp", bufs=3) as xp,
            tc.tile_pool(name="yp", bufs=2) as yp,
            tc.tile_pool(name="ep", bufs=5) as ep,
            tc.tile_pool(name="tp", bufs=4) as tp,
            tc.tile_pool(name="ps", bufs=2, space="PSUM") as psp,
        ):
            w9 = wp.tile([128, 32, 128], f16)
            nc.sync.dma_start(w9[:], w9d.ap()[:])
            wd = wp.tile([128, 64, 128], f16)
            nc.sync.dma_start(wd[:], wdd.ap()[:])
            tg = wp.tile([128, 16], f32)
            nc.sync.dma_start(tg[:], trig.ap()[:])

            if static_x:
                xs = wp.tile([128, 8192], f16)
                nc.sync.dma_start(xs[:], xin.ap()[0][:, :])

            for g in [g for _ in range(reps) for g in range(n_groups)]:
                if static_x:
                    xt = xs
                    if dummy_load:
                        xd = xp.tile([128, 8192], f16, name="xd")
                        nc.sync.dma_start(xd[:, 0:4096], xin.ap()[g][:, 0:4096])
                        nc.sync.dma_start(xd[:, 4096:8192], xin.ap()[g][:, 4096:8192])
                else:
                    xt = xp.tile([128, 8192], f16)
                    nc.sync.dma_start(xt[:, 0:4096], xin.ap()[g][:, 0:4096])
                    nc.sync.dma_start(xt[:, 4096:8192], xin.ap()[g][:, 4096:8192])
                yt = yp.tile([128, 8192], f16)
                for half in (0, 1):
                    cgs = (2 * half, 2 * half + 1)
                    ps, ev = {}, {}
                    # PE: both pairs' matmuls
                    for cg in cgs:
                        p = psp.tile([128, 2048], f32, tag="ps")
                        ps[cg] = p
                        if pair_modes[cg] == "dense":
                            mm_pair(p, wd, cg, 8, xt)
                        else:
                            mm_pair(p, w9, cg, 4, xt)
                    if stage_n == 0:
                        continue
                    # ACT: one FD=2048 eviction per pair
                    for cg in cgs:
                        ysl = yt[:, 2 * cg * 1024 : (2 * cg + 2) * 1024]
                        if pair_modes[cg] == "dense":
                            nc.scalar.copy(ysl, ps[cg][:])
                        else:
                            e = ep.tile([128, 2048], f16, tag="e")
                            nc.scalar.copy(e[:], ps[cg][:])
                            ev[cg] = e
                    if stage_n == 1:
                        continue
                    # DVE: shear waves interleaved across the half's pairs
                    sh = [cg for cg in cgs if pair_modes[cg] == "shear"]
                    u1 = {}
                    for cg in sh:  # wave 1: u1 = a9.E_hi + E_lo
                        u1[cg] = tp.tile([128, 1024], f16, tag="t", name=f"u1_{cg}")
                        nc.vector.scalar_tensor_tensor(
                            u1[cg][:], ev[cg][:, 1024:2048],
                            tg[:, 0 + cg : 1 + cg], ev[cg][:, 0:1024], mult, add,
                        )
                    for cg in sh:  # wave 2: y_hi = s9'.u1 + E_hi
                        nc.vector.scalar_tensor_tensor(
                            yt[:, (2 * cg + 1) * 1024 : (2 * cg + 2) * 1024],
                            u1[cg][:], tg[:, 4 + cg : 5 + cg],
                            ev[cg][:, 1024:2048], mult, add,
                        )
                    for cg in sh:  # wave 3: y_lo = a9.y_hi + u1
                        nc.vector.scalar_tensor_tensor(
                            yt[:, 2 * cg * 1024 : (2 * cg + 1) * 1024],
                            yt[:, (2 * cg + 1) * 1024 : (2 * cg + 2) * 1024],
                            tg[:, 0 + cg : 1 + cg], u1[cg][:], mult, add,
                        )
                    if stage_n >= 3:
                        nc.gpsimd.dma_start(
                            yout.ap()[g][:, half * 4096 : (half + 1) * 4096],
                            yt[:, half * 4096 : (half + 1) * 4096],
                        )
    _split_multi_waits(nc)
    return nc


_CACHE = {}


def _get_nc(n_groups=N_GROUPS):
    if n_groups not in _CACHE:
        _CACHE[n_groups] = build_bass(n_groups)
    return _CACHE[n_groups]


def make_in_maps(x, angles):
    """Pack full inputs into per-core in_maps (list of dicts)."""
    x = np.asarray(x, dtype=np.float32)
    angles = np.asarray(angles, dtype=np.float32)
    w9, wd, trig = _host_tables(angles)
    flat = x.reshape(-1, DIM).astype(np.float16)
    in_maps = []
    for k in range(N_CORES):
        shard = flat[k * ROWS_PER_CORE : (k + 1) * ROWS_PER_CORE]
        in_maps.append({"xin": _pack_x(shard), "w9": w9, "wd": wd, "trig": trig})
    return in_maps


def kernel(x, angles):
    from concourse.bass_utils import run_bass_kernel_spmd

    x = np.asarray(x)
    orig_shape = x.shape
    in_maps = make_in_maps(x, angles)
    nc = _get_nc()
    res = run_bass_kernel_spmd(nc, in_maps, core_ids=list(range(N_CORES)))
    parts = [_unpack_y(res.results[k]["yout"]) for k in range(N_CORES)]
    out = np.concatenate(parts, axis=0).reshape(orig_shape)
    return out.astype(np.float32)



# revision 4
# speedup vs baseline: 1.1144x; 1.1144x over previous
"""Butterfly rotation (10 stages, DIM=1024) on 8 Trainium2 NeuronCores.

Math: the 10-stage butterfly is linear.  Stages 0..8 mix within 512-wide
halves; stages 7/8/9 are, per dim-within-chunk p, rotations between whole
128-wide chunks with per-p angles.  Engine roofs per core (measured):
DMA ~96 us for the fp16 16 MiB in + 16 MiB out round trip (load-only is
~37 us, so the limit is bidirectional); PE fp16 matmul is 1 row/cycle at
2.4 GHz (213 ns per [128,128,512] MM); DVE stt [128,1024] fp16 ~815 ns;
ACT PSUM->SBUF copy ~731 ns.

Scheme (keeps every engine under the ~96 us DMA roofline):
  - Output chunks 0..3 ("path B"): PE applies stages 0..7 only (each
    output chunk depends on one 256-wide block = 2 input chunks), with
    kappa = cos(th8)*cos(th9) folded into the weight rows.  Stage 8 is
    then 2 DVE stt per chunk pair using coefficients t8*c9A/c9B.
  - Output chunks 4..7 ("path A"): PE applies stages 0..8 (4 input
    chunks), with cos(th9) folded in.
  - Stage 9 for all pairs (cg, cg+4) is 2 DVE stt: y_lo = q'lo - t9*q'hi,
    y_hi = t9*q'lo + q'hi, where q' = c9*q comes out of PE/stage-8 with
    the cos pre-folded.  The apparent 1/cos blowup cancels exactly: every
    stored term carries the same cos factor its coefficient divides by.
  PE/group: 4*2*2 + 4*4*2 = 48 MM (vs 80 dense) -> ~82 us; DVE: 12 stt
  -> ~78 us; ACT: 8 evictions -> ~47 us; all under DMA ~96 us.

Device layout (per core, 8192 rows; pure data parallelism): host packs
dim-major fp16: xin[g, p, c*1024 + r] = x[g*1024 + r, c*128 + p] (g: 8
row-groups of 1024 rows, c: 8 dim-chunks of 128, p: dim-within-chunk).
Output uses the same layout (slot = chunk).  Host inverse-permutes and
upcasts the fp16 output.
"""

import os
import sys

sys.path.insert(0, "/opt/trn_rl_repo")

# run_bass_kernel_spmd would try to import the (absent) axon NTFF hook if
# BASS_TRACE is set in the environment.
os.environ["BASS_NEVER_TRACE"] = "1"

import numpy as np

DIM = 1024
STAGES = 10
N_CORES = 8
ROWS_PER_CORE = 8192
GROUP_ROWS = 1024
N_GROUPS = ROWS_PER_CORE // GROUP_ROWS  # 8

# stage-8 DVE pairs (path B): chunk pairs (A, A+2) with their theta8 slice
S8_PAIRS = [(0, 2, 0), (1, 3, 128)]  # (A, B, th8 offset)


def _stage_idx(dim, stage):
    stride = 2**stage
    idx_i = np.arange(dim).reshape(-1, 2 * stride)[:, :stride].ravel()
    idx_j = idx_i + stride
    return idx_i, idx_j


def _butterfly_apply(v, angles, stages):
    """Apply butterfly stages to rows of v (float64, in place) and return v."""
    for s in stages:
        idx_i, idx_j = _stage_idx(v.shape[1], s)
        c = np.cos(angles[s].astype(np.float64))
        sn = np.sin(angles[s].astype(np.float64))
        vi = v[:, idx_i].copy()
        vj = v[:, idx_j].copy()
        v[:, idx_i] = c * vi - sn * vj
        v[:, idx_j] = sn * vi + c * vj
    return v


def _host_tables(angles):
    """wb[k, i, m] fp16 lhsT blocks (24 of them):
      i = c*2 + t        (c in 0..3, t in 0..1): path-B block, input chunk
                         ci = 2*(c//2) + t, rows scaled by c8(c)*c9[c%4]
      i = 8 + (c-4)*4+t  (c in 4..7, t in 0..3): path-A block, input chunk
                         ci = 4 + t, rows scaled by c9[c%4]
    trig[p, j] f32: j=0..3 t9[cg]; 4..7 -t9[cg]; 8,9 pair(0,2) coefA/coefB;
    10,11 pair(1,3) coefA/coefB.
    """
    th = angles.astype(np.float64)
    # _butterfly_apply on eye gives mb[i, j] = M[j, i] (M maps in->out),
    # so lhsT[k, m] = M[c*128+m, ci*128+k] = mb[ci*128+k, c*128+m].
    mb7 = _butterfly_apply(np.eye(DIM, dtype=np.float64), angles, range(8))
    mb8 = _butterfly_apply(np.eye(DIM, dtype=np.float64), angles, range(9))

    c9 = [np.cos(th[9][cg * 128 : (cg + 1) * 128]) for cg in range(4)]
    s9 = [np.sin(th[9][cg * 128 : (cg + 1) * 128]) for cg in range(4)]
    # stage-8 angle slices: pair (0,2)->th8[0:128], (1,3)->th8[128:256]
    c8 = {0: np.cos(th[8][0:128]), 1: np.cos(th[8][128:256])}
    s8 = {0: np.sin(th[8][0:128]), 1: np.sin(th[8][128:256])}

    wb = np.empty((128, 24, 128), dtype=np.float16)
    for c in range(4):  # path B
        kap = c8[c % 2] * c9[c % 4]
        for t in range(2):
            ci = 2 * (c // 2) + t
            blk = mb7[ci * 128 : (ci + 1) * 128, c * 128 : (c + 1) * 128]
            wb[:, c * 2 + t, :] = (blk * kap[None, :]).astype(np.float16)
    for c in range(4, 8):  # path A
        kap = c9[c % 4]
        for t in range(4):
            ci = 4 + t
            blk = mb8[ci * 128 : (ci + 1) * 128, c * 128 : (c + 1) * 128]
            wb[:, 8 + (c - 4) * 4 + t, :] = (blk * kap[None, :]).astype(np.float16)

    trig = np.empty((128, 12), dtype=np.float32)
    for cg in range(4):
        t9 = s9[cg] / c9[cg]
        trig[:, cg] = t9
        trig[:, 4 + cg] = -t9
    for j, (A, B, off) in enumerate(S8_PAIRS):
        t8 = s8[j] / c8[j]
        trig[:, 8 + 2 * j] = -t8 * c9[A % 4] / c9[B % 4]  # coefA
        trig[:, 9 + 2 * j] = t8 * c9[B % 4] / c9[A % 4]  # coefB
    return wb, trig


def _pack_x(x_core, n_groups=N_GROUPS):
    # [G*1024, 1024] -> [G, 128, 8192] with xin[g, p, c*1024+r] = x[g*1024+r, c*128+p]
    g = x_core.reshape(n_groups, GROUP_ROWS, 8, 128)
    return np.ascontiguousarray(
        g.transpose(0, 3, 2, 1).reshape(n_groups, 128, 8 * GROUP_ROWS)
    )


def _unpack_y(y_packed, n_groups=N_GROUPS):
    # yout[g, p, c*1024 + r] = y[g*1024 + r, c*128 + p]  (slot = chunk)
    g = y_packed.reshape(n_groups, 128, 8, GROUP_ROWS)
    g = g.transpose(0, 3, 2, 1)  # [g, r, c, p]
    return np.ascontiguousarray(g.reshape(n_groups * GROUP_ROWS, DIM))


def _patch_tile_drain():
    """Workaround: this walrus build cannot encode semaphore waits on a
    sequencer Drain/NoOp with >1 wait ("Too many sync wait commands").
    Re-emit the TileContext tail waits as one nop per semaphore."""
    from concourse import mybir, tile
    from concourse.vector_clock import ScopedClock

    if getattr(tile.TileContext, "_drain_patched", False):
        return

    def _drain_and_barrier(self, tick_clock, wait_clock):
        nop_inst = self.nc.sync.nop(nofuse=True)
        wait_clock.add_sem_waits(
            nop_inst.ins, ScopedClock({None: tick_clock.global_clock})
        )
        si = nop_inst.ins.sync_info
        if si is not None and si.on_wait and len(si.on_wait) > 1:
            extra = si.on_wait[1:]
            si.on_wait = si.on_wait[:1]
            for w in extra:
                extra_nop = self.nc.sync.nop(nofuse=True)
                esi = extra_nop.ins.sync_info
                if esi is None:
                    extra_nop.ins.sync_info = mybir.SyncInfo(on_wait=[w], on_update=[])
                else:
                    esi.on_wait = list(esi.on_wait or []) + [w]
        self.nc.sync.drain()
        self.nc.all_engine_barrier()
        assert self.sems is not None
        popped = self.nc._tile_sem_poison_stack.pop()
        assert popped is self._sem_poison
        self.nc.clear_and_free_semaphores(list(self.sems.allocated().values()))
        self.nc.all_engine_barrier()

    tile.TileContext._drain_and_barrier = _drain_and_barrier
    tile.TileContext._drain_patched = True


def _split_multi_waits(nc, limit=1):
    """This walrus build encodes at most `limit` semaphore wait(s) per
    instruction ("Too many sync wait commands").  Hoist excess waits onto
    same-engine NoOps inserted immediately before the instruction."""
    from concourse import mybir

    counter = [0]

    def fresh_nop(engine, waits):
        counter[0] += 1
        nop = mybir.InstNoOp(
            name=f"waitsplit-{counter[0]}",
            engine=engine,
            ins=[],
            outs=[],
            bass_nofuse=True,
            sync_info=mybir.SyncInfo(on_wait=list(waits), on_update=[]),
        )
        nc.register_instruction(nop, overwrite=True)
        return nop

    for fn in nc.m.functions:
        for bb in fn.blocks:
            changed = False
            new = []
            for inst in bb.instructions:
                si = getattr(inst, "sync_info", None)
                if si is not None and si.on_wait and len(si.on_wait) > limit:
                    extra = si.on_wait[: len(si.on_wait) - limit]
                    si.on_wait = si.on_wait[len(si.on_wait) - limit :]
                    for k in range(0, len(extra), limit):
                        new.append(fresh_nop(inst.engine, extra[k : k + limit]))
                    changed = True
                new.append(inst)
            if changed:
                bb.instructions = new
    return nc


def build_bass(n_groups=N_GROUPS, reps=1, upto="full"):
    """Build the Bass module for one core processing n_groups row-groups.
    reps>1 repeats the whole pipeline in-NEFF (for timing calibration).
    upto: 'pe' | 'evict' | 'dve' | 'full' truncates the pipeline (for
    engine-attribution benchmarks)."""
    _patch_tile_drain()
    from concourse import bass, mybir, tile

    stage_n = ["pe", "evict", "dve", "full"].index(upto)
    f16 = mybir.dt.float16
    f32 = mybir.dt.float32
    nc = bass.Bass("TRN2", target_bir_lowering=False, debug=False)
    xin = nc.dram_tensor("xin", [n_groups, 128, 8192], f16, kind="ExternalInput")
    wbd = nc.dram_tensor("wb", [128, 24, 128], f16, kind="ExternalInput")
    trig = nc.dram_tensor("trig", [128, 12], f32, kind="ExternalInput")
    yout = nc.dram_tensor("yout", [n_groups, 128, 8192], f16, kind="ExternalOutput")

    mult = mybir.AluOpType.mult
    add = mybir.AluOpType.add

    # PE emission order: path-B stage-8 partners adjacent, then path-A in
    # the order the stage-9 waves consume them.
    PE_ORDER = [0, 2, 1, 3, 4, 6, 5, 7]

    def mm_chunk(psum, wtile, c, xt):
        """Accumulate output chunk c's [128, 1024] PSUM tile."""
        if c < 4:
            idxs = [c * 2 + t for t in range(2)]
            cis = [2 * (c // 2) + t for t in range(2)]
        else:
            idxs = [8 + (c - 4) * 4 + t for t in range(4)]
            cis = [4 + t for t in range(4)]
        for h in range(2):
            for j, (i, ci) in enumerate(zip(idxs, cis)):
                nc.tensor.matmul(
                    psum[:, h * 512 : (h + 1) * 512],
                    wtile[:, i, :],
                    xt[:, ci * 1024 + h * 512 : ci * 1024 + (h + 1) * 512],
                    start=(j == 0),
                    stop=(j == len(idxs) - 1),
                )

    with tile.TileContext(nc) as tc:
        with (
            tc.tile_pool(name="wp", bufs=1) as wp,
            tc.tile_pool(name="xp", bufs=3) as xp,
            tc.tile_pool(name="yp", bufs=2) as yp,
            tc.tile_pool(name="ep", bufs=12) as ep,
            tc.tile_pool(name="qp", bufs=6) as qp,
            tc.tile_pool(name="ps", bufs=4, space="PSUM") as psp,
        ):
            wb = wp.tile([128, 24, 128], f16)
            nc.sync.dma_start(wb[:], wbd.ap()[:])
            tg = wp.tile([128, 12], f32)
            nc.sync.dma_start(tg[:], trig.ap()[:])

            for g in [g for _ in range(reps) for g in range(n_groups)]:
                xt = xp.tile([128, 8192], f16)
                nc.sync.dma_start(xt[:, 0:4096], xin.ap()[g][:, 0:4096])
                nc.sync.dma_start(xt[:, 4096:8192], xin.ap()[g][:, 4096:8192])
                yt = yp.tile([128, 8192], f16)

                # PE + ACT eviction per chunk
                w = {}
                for c in PE_ORDER:
                    p = psp.tile([128, 1024], f32, tag="ps")
                    mm_chunk(p, wb, c, xt)
                    if stage_n >= 1:
                        e = ep.tile([128, 1024], f16, tag="e", name=f"w{c}")
                        nc.scalar.copy(e[:], p[:])
                        w[c] = e
                if stage_n < 2:
                    continue

                # DVE stage 8 (path-B pairs): q'A = coefA*wB + wA, etc.
                q = {}
                for j, (A, B, _) in enumerate(S8_PAIRS):
                    qA = qp.tile([128, 1024], f16, tag="q", name=f"q{A}")
                    nc.vector.scalar_tensor_tensor(
                        qA[:], w[B][:], tg[:, 8 + 2 * j : 9 + 2 * j], w[A][:],
                        mult, add,
                    )
                    qB = qp.tile([128, 1024], f16, tag="q", name=f"q{B}")
                    nc.vector.scalar_tensor_tensor(
                        qB[:], w[A][:], tg[:, 9 + 2 * j : 10 + 2 * j], w[B][:],
                        mult, add,
                    )
                    q[A], q[B] = qA, qB
                for c in (4, 5, 6, 7):
                    q[c] = w[c]

                # DVE stage 9: pairs (cg, cg+4) into the output tile
                for cg in (0, 2, 1, 3):
                    lo = yt[:, cg * 1024 : (cg + 1) * 1024]
                    hi = yt[:, (cg + 4) * 1024 : (cg + 5) * 1024]
                    nc.vector.scalar_tensor_tensor(
                        lo, q[cg + 4][:], tg[:, 4 + cg : 5 + cg], q[cg][:],
                        mult, add,
                    )
                    nc.vector.scalar_tensor_tensor(
                        hi, q[cg][:], tg[:, cg : cg + 1], q[cg + 4][:],
                        mult, add,
                    )
                if stage_n < 3:
                    continue
                # stores ride the ACT HWDGE ring: loads-on-SP + stores-on-ACT
                # measured 86 us for the round trip vs 96 us with SWDGE stores
                nc.scalar.dma_start(yout.ap()[g][:, 0:4096], yt[:, 0:4096])
                nc.scalar.dma_start(yout.ap()[g][:, 4096:8192], yt[:, 4096:8192])
    _split_multi_waits(nc)
    return nc


_CACHE = {}


def _get_nc(n_groups=N_GROUPS):
    if n_groups not in _CACHE:
        _CACHE[n_groups] = build_bass(n_groups)
    return _CACHE[n_groups]


def make_in_maps(x, angles):
    """Pack full inputs into per-core in_maps (list of dicts)."""
    x = np.asarray(x, dtype=np.float32)
    angles = np.asarray(angles, dtype=np.float32)
    wb, trig = _host_tables(angles)
    flat = x.reshape(-1, DIM).astype(np.float16)
    in_maps = []
    for k in range(N_CORES):
        shard = flat[k * ROWS_PER_CORE : (k + 1) * ROWS_PER_CORE]
        in_maps.append({"xin": _pack_x(shard), "wb": wb, "trig": trig})
    return in_maps


def kernel(x, angles):
    from concourse.bass_utils import run_bass_kernel_spmd

    x = np.asarray(x)
    orig_shape = x.shape
    in_maps = make_in_maps(x, angles)
    nc = _get_nc()
    res = run_bass_kernel_spmd(nc, in_maps, core_ids=list(range(N_CORES)))
    parts = [_unpack_y(res.results[k]["yout"]) for k in range(N_CORES)]
    out = np.concatenate(parts, axis=0).reshape(orig_shape)
    return out.astype(np.float32)


# revision 5
# speedup vs baseline: 1.1770x; 1.0562x over previous
"""Butterfly rotation (10 stages, DIM=1024) on 8 Trainium2 NeuronCores.

Math: the 10-stage butterfly is linear.  Stages 0..8 mix within 512-wide
halves; stages 7/8/9 are, per dim-within-chunk p, rotations between whole
128-wide chunks with per-p angles.  Engine roofs per core (measured):
DMA ~96 us for the fp16 16 MiB in + 16 MiB out round trip (load-only is
~37 us, so the limit is bidirectional); PE fp16 matmul is 1 row/cycle at
2.4 GHz (213 ns per [128,128,512] MM); DVE stt [128,1024] fp16 ~815 ns;
ACT PSUM->SBUF copy ~731 ns.

Scheme (keeps every engine under the ~96 us DMA roofline):
  - Output chunks 0..3 ("path B"): PE applies stages 0..7 only (each
    output chunk depends on one 256-wide block = 2 input chunks), with
    kappa = cos(th8)*cos(th9) folded into the weight rows.  Stage 8 is
    then 2 DVE stt per chunk pair using coefficients t8*c9A/c9B.
  - Output chunks 4..7 ("path A"): PE applies stages 0..8 (4 input
    chunks), with cos(th9) folded in.
  - Stage 9 for all pairs (cg, cg+4) is 2 DVE stt: y_lo = q'lo - t9*q'hi,
    y_hi = t9*q'lo + q'hi, where q' = c9*q comes out of PE/stage-8 with
    the cos pre-folded.  The apparent 1/cos blowup cancels exactly: every
    stored term carries the same cos factor its coefficient divides by.
  PE/group: 4*2*2 + 4*4*2 = 48 MM (vs 80 dense) -> ~82 us; DVE: 12 stt
  -> ~78 us; ACT: 8 evictions -> ~47 us; all under DMA ~96 us.

Device layout (per core, 8192 rows; pure data parallelism): host packs
dim-major fp16: xin[g, p, c*1024 + r] = x[g*1024 + r, c*128 + p] (g: 8
row-groups of 1024 rows, c: 8 dim-chunks of 128, p: dim-within-chunk).
Output uses the same layout (slot = chunk).  Host inverse-permutes and
upcasts the fp16 output.
"""

import os
import sys

sys.path.insert(0, "/opt/trn_rl_repo")

# run_bass_kernel_spmd would try to import the (absent) axon NTFF hook if
# BASS_TRACE is set in the environment.
os.environ["BASS_NEVER_TRACE"] = "1"

import numpy as np

DIM = 1024
STAGES = 10
N_CORES = 8
ROWS_PER_CORE = 8192
GROUP_ROWS = 1024
N_GROUPS = ROWS_PER_CORE // GROUP_ROWS  # 8

# stage-8 DVE pairs (path B): chunk pairs (A, A+2) with their theta8 slice
S8_PAIRS = [(0, 2, 0), (1, 3, 128)]  # (A, B, th8 offset)


def _stage_idx(dim, stage):
    stride = 2**stage
    idx_i = np.arange(dim).reshape(-1, 2 * stride)[:, :stride].ravel()
    idx_j = idx_i + stride
    return idx_i, idx_j


def _butterfly_apply(v, angles, stages):
    """Apply butterfly stages to rows of v (float64, in place) and return v."""
    for s in stages:
        idx_i, idx_j = _stage_idx(v.shape[1], s)
        c = np.cos(angles[s].astype(np.float64))
        sn = np.sin(angles[s].astype(np.float64))
        vi = v[:, idx_i].copy()
        vj = v[:, idx_j].copy()
        v[:, idx_i] = c * vi - sn * vj
        v[:, idx_j] = sn * vi + c * vj
    return v


def _host_tables(angles):
    """wb[k, i, m] fp16 lhsT blocks (24 of them):
      i = c*2 + t        (c in 0..3, t in 0..1): path-B block, input chunk
                         ci = 2*(c//2) + t, rows scaled by c8(c)*c9[c%4]
      i = 8 + (c-4)*4+t  (c in 4..7, t in 0..3): path-A block, input chunk
                         ci = 4 + t, rows scaled by c9[c%4]
    trig[p, j] f32: j=0..3 t9[cg]; 4..7 -t9[cg]; 8,9 pair(0,2) coefA/coefB;
    10,11 pair(1,3) coefA/coefB.
    """
    th = angles.astype(np.float64)
    # _butterfly_apply on eye gives mb[i, j] = M[j, i] (M maps in->out),
    # so lhsT[k, m] = M[c*128+m, ci*128+k] = mb[ci*128+k, c*128+m].
    mb7 = _butterfly_apply(np.eye(DIM, dtype=np.float64), angles, range(8))
    mb8 = _butterfly_apply(np.eye(DIM, dtype=np.float64), angles, range(9))

    c9 = [np.cos(th[9][cg * 128 : (cg + 1) * 128]) for cg in range(4)]
    s9 = [np.sin(th[9][cg * 128 : (cg + 1) * 128]) for cg in range(4)]
    # stage-8 angle slices: pair (0,2)->th8[0:128], (1,3)->th8[128:256]
    c8 = {0: np.cos(th[8][0:128]), 1: np.cos(th[8][128:256])}
    s8 = {0: np.sin(th[8][0:128]), 1: np.sin(th[8][128:256])}

    wb = np.empty((128, 24, 128), dtype=np.float16)
    for c in range(4):  # path B
        kap = c8[c % 2] * c9[c % 4]
        for t in range(2):
            ci = 2 * (c // 2) + t
            blk = mb7[ci * 128 : (ci + 1) * 128, c * 128 : (c + 1) * 128]
            wb[:, c * 2 + t, :] = (blk * kap[None, :]).astype(np.float16)
    for c in range(4, 8):  # path A
        kap = c9[c % 4]
        for t in range(4):
            ci = 4 + t
            blk = mb8[ci * 128 : (ci + 1) * 128, c * 128 : (c + 1) * 128]
            wb[:, 8 + (c - 4) * 4 + t, :] = (blk * kap[None, :]).astype(np.float16)

    trig = np.empty((128, 12), dtype=np.float32)
    for cg in range(4):
        t9 = s9[cg] / c9[cg]
        trig[:, cg] = t9
        trig[:, 4 + cg] = -t9
    for j, (A, B, off) in enumerate(S8_PAIRS):
        t8 = s8[j] / c8[j]
        trig[:, 8 + 2 * j] = -t8 * c9[A % 4] / c9[B % 4]  # coefA
        trig[:, 9 + 2 * j] = t8 * c9[B % 4] / c9[A % 4]  # coefB
    return wb, trig


def _pack_x(x_core, n_groups=N_GROUPS):
    # [G*1024, 1024] -> [G, 128, 8192] with xin[g, p, c*1024+r] = x[g*1024+r, c*128+p]
    g = x_core.reshape(n_groups, GROUP_ROWS, 8, 128)
    return np.ascontiguousarray(
        g.transpose(0, 3, 2, 1).reshape(n_groups, 128, 8 * GROUP_ROWS)
    )


def _unpack_y(y_packed, n_groups=N_GROUPS):
    # yout[g, p, c*1024 + r] = y[g*1024 + r, c*128 + p]  (slot = chunk)
    g = y_packed.reshape(n_groups, 128, 8, GROUP_ROWS)
    g = g.transpose(0, 3, 2, 1)  # [g, r, c, p]
    return np.ascontiguousarray(g.reshape(n_groups * GROUP_ROWS, DIM))


def _patch_tile_drain():
    """Workaround: this walrus build cannot encode semaphore waits on a
    sequencer Drain/NoOp with >1 wait ("Too many sync wait commands").
    Re-emit the TileContext tail waits as one nop per semaphore."""
    from concourse import mybir, tile
    from concourse.vector_clock import ScopedClock

    if getattr(tile.TileContext, "_drain_patched", False):
        return

    def _drain_and_barrier(self, tick_clock, wait_clock):
        nop_inst = self.nc.sync.nop(nofuse=True)
        wait_clock.add_sem_waits(
            nop_inst.ins, ScopedClock({None: tick_clock.global_clock})
        )
        si = nop_inst.ins.sync_info
        if si is not None and si.on_wait and len(si.on_wait) > 1:
            extra = si.on_wait[1:]
            si.on_wait = si.on_wait[:1]
            for w in extra:
                extra_nop = self.nc.sync.nop(nofuse=True)
                esi = extra_nop.ins.sync_info
                if esi is None:
                    extra_nop.ins.sync_info = mybir.SyncInfo(on_wait=[w], on_update=[])
                else:
                    esi.on_wait = list(esi.on_wait or []) + [w]
        self.nc.sync.drain()
        self.nc.all_engine_barrier()
        assert self.sems is not None
        popped = self.nc._tile_sem_poison_stack.pop()
        assert popped is self._sem_poison
        self.nc.clear_and_free_semaphores(list(self.sems.allocated().values()))
        self.nc.all_engine_barrier()

    tile.TileContext._drain_and_barrier = _drain_and_barrier
    tile.TileContext._drain_patched = True


def _split_multi_waits(nc, limit=1):
    """This walrus build encodes at most `limit` semaphore wait(s) per
    instruction ("Too many sync wait commands").  Hoist excess waits onto
    same-engine NoOps inserted immediately before the instruction."""
    from concourse import mybir

    counter = [0]

    def fresh_nop(engine, waits):
        counter[0] += 1
        nop = mybir.InstNoOp(
            name=f"waitsplit-{counter[0]}",
            engine=engine,
            ins=[],
            outs=[],
            bass_nofuse=True,
            sync_info=mybir.SyncInfo(on_wait=list(waits), on_update=[]),
        )
        nc.register_instruction(nop, overwrite=True)
        return nop

    for fn in nc.m.functions:
        for bb in fn.blocks:
            changed = False
            new = []
            for inst in bb.instructions:
                si = getattr(inst, "sync_info", None)
                if si is not None and si.on_wait and len(si.on_wait) > limit:
                    extra = si.on_wait[: len(si.on_wait) - limit]
                    si.on_wait = si.on_wait[len(si.on_wait) - limit :]
                    for k in range(0, len(extra), limit):
                        new.append(fresh_nop(inst.engine, extra[k : k + limit]))
                    changed = True
                new.append(inst)
            if changed:
                bb.instructions = new
    return nc


def build_bass(n_groups=N_GROUPS, reps=1, upto="full"):
    """Build the Bass module for one core processing n_groups row-groups.
    reps>1 repeats the whole pipeline in-NEFF (for timing calibration).
    upto: 'pe' | 'evict' | 'dve' | 'full' truncates the pipeline (for
    engine-attribution benchmarks)."""
    _patch_tile_drain()
    from concourse import bass, mybir, tile

    stage_n = ["pe", "evict", "dve", "full"].index(upto)
    f16 = mybir.dt.float16
    f32 = mybir.dt.float32
    nc = bass.Bass("TRN2", target_bir_lowering=False, debug=False)
    xin = nc.dram_tensor("xin", [n_groups, 128, 8192], f16, kind="ExternalInput")
    wbd = nc.dram_tensor("wb", [128, 24, 128], f16, kind="ExternalInput")
    trig = nc.dram_tensor("trig", [128, 12], f32, kind="ExternalInput")
    yout = nc.dram_tensor("yout", [n_groups, 128, 8192], f16, kind="ExternalOutput")

    mult = mybir.AluOpType.mult
    add = mybir.AluOpType.add

    # PE emission order: path-B stage-8 partners adjacent, then path-A in
    # the order the stage-9 waves consume them.
    PE_ORDER = [0, 2, 1, 3, 4, 6, 5, 7]

    def mm_chunk(psum, wtile, c, xt):
        """Accumulate output chunk c's [128, 1024] PSUM tile."""
        if c < 4:
            idxs = [c * 2 + t for t in range(2)]
            cis = [2 * (c // 2) + t for t in range(2)]
        else:
            idxs = [8 + (c - 4) * 4 + t for t in range(4)]
            cis = [4 + t for t in range(4)]
        for h in range(2):
            for j, (i, ci) in enumerate(zip(idxs, cis)):
                nc.tensor.matmul(
                    psum[:, h * 512 : (h + 1) * 512],
                    wtile[:, i, :],
                    xt[:, ci * 1024 + h * 512 : ci * 1024 + (h + 1) * 512],
                    start=(j == 0),
                    stop=(j == len(idxs) - 1),
                )

    with tile.TileContext(nc) as tc:
        with (
            tc.tile_pool(name="wp", bufs=1) as wp,
            tc.tile_pool(name="xp", bufs=3) as xp,
            tc.tile_pool(name="yp", bufs=2) as yp,
            tc.tile_pool(name="ep", bufs=12) as ep,
            tc.tile_pool(name="qp", bufs=6) as qp,
            tc.tile_pool(name="ps", bufs=4, space="PSUM") as psp,
        ):
            wb = wp.tile([128, 24, 128], f16)
            nc.sync.dma_start(wb[:], wbd.ap()[:])
            tg = wp.tile([128, 12], f32)
            nc.sync.dma_start(tg[:], trig.ap()[:])

            for g in [g for _ in range(reps) for g in range(n_groups)]:
                xt = xp.tile([128, 8192], f16)
                nc.sync.dma_start(xt[:, 0:4096], xin.ap()[g][:, 0:4096])
                nc.sync.dma_start(xt[:, 4096:8192], xin.ap()[g][:, 4096:8192])
                yt = yp.tile([128, 8192], f16)

                # PE + ACT eviction per chunk
                w = {}
                for c in PE_ORDER:
                    p = psp.tile([128, 1024], f32, tag="ps")
                    mm_chunk(p, wb, c, xt)
                    if stage_n >= 1:
                        e = ep.tile([128, 1024], f16, tag="e", name=f"w{c}")
                        nc.scalar.copy(e[:], p[:])
                        w[c] = e
                if stage_n < 2:
                    continue

                # DVE stage 8 (path-B pairs): q'A = coefA*wB + wA, etc.
                q = {}
                for j, (A, B, _) in enumerate(S8_PAIRS):
                    qA = qp.tile([128, 1024], f16, tag="q", name=f"q{A}")
                    nc.vector.scalar_tensor_tensor(
                        qA[:], w[B][:], tg[:, 8 + 2 * j : 9 + 2 * j], w[A][:],
                        mult, add,
                    )
                    qB = qp.tile([128, 1024], f16, tag="q", name=f"q{B}")
                    nc.vector.scalar_tensor_tensor(
                        qB[:], w[A][:], tg[:, 9 + 2 * j : 10 + 2 * j], w[B][:],
                        mult, add,
                    )
                    q[A], q[B] = qA, qB
                for c in (4, 5, 6, 7):
                    q[c] = w[c]

                # DVE stage 9: pairs (cg, cg+4) into the output tile.  All
                # lo outputs (slots 0..3) first so their stores fire while
                # the hi waves still run.
                for cg in range(4):
                    nc.vector.scalar_tensor_tensor(
                        yt[:, cg * 1024 : (cg + 1) * 1024],
                        q[cg + 4][:], tg[:, 4 + cg : 5 + cg], q[cg][:],
                        mult, add,
                    )
                    if stage_n >= 3 and cg % 2 == 1:
                        nc.scalar.dma_start(
                            yout.ap()[g][:, (cg - 1) * 1024 : (cg + 1) * 1024],
                            yt[:, (cg - 1) * 1024 : (cg + 1) * 1024],
                        )
                for cg in range(4):
                    nc.vector.scalar_tensor_tensor(
                        yt[:, (cg + 4) * 1024 : (cg + 5) * 1024],
                        q[cg][:], tg[:, cg : cg + 1], q[cg + 4][:],
                        mult, add,
                    )
                    if stage_n >= 3 and cg % 2 == 1:
                        # stores ride the ACT HWDGE ring: loads-on-SP +
                        # stores-on-ACT measured 86 us round trip vs 96 us
                        # with SWDGE stores
                        nc.scalar.dma_start(
                            yout.ap()[g][:, (cg + 3) * 1024 : (cg + 5) * 1024],
                            yt[:, (cg + 3) * 1024 : (cg + 5) * 1024],
                        )
    _split_multi_waits(nc)
    return nc


_CACHE = {}


def _get_nc(n_groups=N_GROUPS):
    if n_groups not in _CACHE:
        _CACHE[n_groups] = build_bass(n_groups)
    return _CACHE[n_groups]


def make_in_maps(x, angles):
    """Pack full inputs into per-core in_maps (list of dicts)."""
    x = np.asarray(x, dtype=np.float32)
    angles = np.asarray(angles, dtype=np.float32)
    wb, trig = _host_tables(angles)
    flat = x.reshape(-1, DIM).astype(np.float16)
    in_maps = []
    for k in range(N_CORES):
        shard = flat[k * ROWS_PER_CORE : (k + 1) * ROWS_PER_CORE]
        in_maps.append({"xin": _pack_x(shard), "wb": wb, "trig": trig})
    return in_maps


def kernel(x, angles):
    from concourse.bass_utils import run_bass_kernel_spmd

    x = np.asarray(x)
    orig_shape = x.shape
    in_maps = make_in_maps(x, angles)
    nc = _get_nc()
    res = run_bass_kernel_spmd(nc, in_maps, core_ids=list(range(N_CORES)))
    parts = [_unpack_y(res.results[k]["yout"]) for k in range(N_CORES)]
    out = np.concatenate(parts, axis=0).reshape(orig_shape)
    return out.astype(np.float32)


# revision 14
# speedup vs baseline: 1.3787x; 1.1714x over previous
"""Butterfly rotation (10 stages, DIM=1024) on 8 Trainium2 NeuronCores.

Math: the 10-stage butterfly is linear.  Stages 0..8 mix within 512-wide
halves; stages 7/8/9 are, per dim-within-chunk p, rotations between whole
128-wide chunks with per-p angles.  Engine roofs per core (measured):
DMA ~96 us for the fp16 16 MiB in + 16 MiB out round trip (load-only is
~37 us, so the limit is bidirectional); PE fp16 matmul is 1 row/cycle at
2.4 GHz (213 ns per [128,128,512] MM); DVE stt [128,1024] fp16 ~815 ns;
ACT PSUM->SBUF copy ~731 ns.

Scheme (keeps every engine under the ~96 us DMA roofline):
  - Output chunks 0..3 ("path B"): PE applies stages 0..7 only (each
    output chunk depends on one 256-wide block = 2 input chunks), with
    kappa = cos(th8)*cos(th9) folded into the weight rows.  Stage 8 is
    then 2 DVE stt per chunk pair using coefficients t8*c9A/c9B.
  - Output chunks 4..7 ("path A"): PE applies stages 0..8 (4 input
    chunks), with cos(th9) folded in.
  - Stage 9 for all pairs (cg, cg+4) is 2 DVE stt: y_lo = q'lo - t9*q'hi,
    y_hi = t9*q'lo + q'hi, where q' = c9*q comes out of PE/stage-8 with
    the cos pre-folded.  The apparent 1/cos blowup cancels exactly: every
    stored term carries the same cos factor its coefficient divides by.
  PE/group: 4*2*2 + 4*4*2 = 48 MM (vs 80 dense) -> ~82 us; DVE: 12 stt
  -> ~78 us; ACT: 8 evictions -> ~47 us; all under DMA ~96 us.

Device layout (per core, 8192 rows; pure data parallelism): host packs
dim-major fp16: xin[g, p, c*1024 + r] = x[g*1024 + r, c*128 + p] (g: 8
row-groups of 1024 rows, c: 8 dim-chunks of 128, p: dim-within-chunk).
Output uses the same layout (slot = chunk).  Host inverse-permutes and
upcasts the fp16 output.
"""

import os
import sys

sys.path.insert(0, "/opt/trn_rl_repo")

# run_bass_kernel_spmd would try to import the (absent) axon NTFF hook if
# BASS_TRACE is set in the environment.
os.environ["BASS_NEVER_TRACE"] = "1"

import numpy as np

DIM = 1024
STAGES = 10
N_CORES = 8
ROWS_PER_CORE = 8192
GROUP_ROWS = 1024
N_GROUPS = ROWS_PER_CORE // GROUP_ROWS  # 8

# stage-8 DVE pairs (path B): chunk pairs (A, A+2) with their theta8 slice
S8_PAIRS = [(0, 2, 0), (1, 3, 128)]  # (A, B, th8 offset)


def _stage_idx(dim, stage):
    stride = 2**stage
    idx_i = np.arange(dim).reshape(-1, 2 * stride)[:, :stride].ravel()
    idx_j = idx_i + stride
    return idx_i, idx_j


def _butterfly_apply(v, angles, stages):
    """Apply butterfly stages to rows of v (float64, in place) and return v."""
    for s in stages:
        idx_i, idx_j = _stage_idx(v.shape[1], s)
        c = np.cos(angles[s].astype(np.float64))
        sn = np.sin(angles[s].astype(np.float64))
        vi = v[:, idx_i].copy()
        vj = v[:, idx_j].copy()
        v[:, idx_i] = c * vi - sn * vj
        v[:, idx_j] = sn * vi + c * vj
    return v


def _host_tables(angles):
    """wb[k, i, m] fp16 lhsT blocks (24 of them):
      i = c*2 + t        (c in 0..3, t in 0..1): path-B block, input chunk
                         ci = 2*(c//2) + t, rows scaled by c8(c)*c9[c%4]
      i = 8 + (c-4)*4+t  (c in 4..7, t in 0..3): path-A block, input chunk
                         ci = 4 + t, rows scaled by c9[c%4]
    trig[p, j] f32: j=0..3 t9[cg]; 4..7 -t9[cg]; 8,9 pair(0,2) coefA/coefB;
    10,11 pair(1,3) coefA/coefB.
    """
    th = angles.astype(np.float64)
    # _butterfly_apply on eye gives mb[i, j] = M[j, i] (M maps in->out),
    # so lhsT[k, m] = M[c*128+m, ci*128+k] = mb[ci*128+k, c*128+m].
    mb7 = _butterfly_apply(np.eye(DIM, dtype=np.float64), angles, range(8))
    mb8 = _butterfly_apply(np.eye(DIM, dtype=np.float64), angles, range(9))

    c9 = [np.cos(th[9][cg * 128 : (cg + 1) * 128]) for cg in range(4)]
    s9 = [np.sin(th[9][cg * 128 : (cg + 1) * 128]) for cg in range(4)]
    # stage-8 angle slices: pair (0,2)->th8[0:128], (1,3)->th8[128:256]
    c8 = {0: np.cos(th[8][0:128]), 1: np.cos(th[8][128:256])}
    s8 = {0: np.sin(th[8][0:128]), 1: np.sin(th[8][128:256])}

    wb = np.empty((128, 24, 128), dtype=np.float16)
    for c in range(4):  # path B
        kap = c8[c % 2] * c9[c % 4]
        for t in range(2):
            ci = 2 * (c // 2) + t
            blk = mb7[ci * 128 : (ci + 1) * 128, c * 128 : (c + 1) * 128]
            wb[:, c * 2 + t, :] = (blk * kap[None, :]).astype(np.float16)
    for c in range(4, 8):  # path A
        kap = c9[c % 4]
        for t in range(4):
            ci = 4 + t
            blk = mb8[ci * 128 : (ci + 1) * 128, c * 128 : (c + 1) * 128]
            wb[:, 8 + (c - 4) * 4 + t, :] = (blk * kap[None, :]).astype(np.float16)

    trig = np.empty((128, 12), dtype=np.float32)
    for cg in range(4):
        t9 = s9[cg] / c9[cg]
        trig[:, cg] = t9
        trig[:, 4 + cg] = -t9
    for j, (A, B, off) in enumerate(S8_PAIRS):
        t8 = s8[j] / c8[j]
        trig[:, 8 + 2 * j] = -t8 * c9[A % 4] / c9[B % 4]  # coefA
        trig[:, 9 + 2 * j] = t8 * c9[B % 4] / c9[A % 4]  # coefB
    return wb, trig


def _pack_x(x_core, n_groups=N_GROUPS):
    # [G*1024, 1024] -> [G, 128, 8192] with xin[g, p, c*1024+r] = x[g*1024+r, c*128+p]
    g = x_core.reshape(n_groups, GROUP_ROWS, 8, 128)
    return np.ascontiguousarray(
        g.transpose(0, 3, 2, 1).reshape(n_groups, 128, 8 * GROUP_ROWS)
    )


def _unpack_y(y_packed, n_groups=N_GROUPS):
    # yout[g, p, c*1024 + r] = y[g*1024 + r, c*128 + p]  (slot = chunk)
    g = y_packed.reshape(n_groups, 128, 8, GROUP_ROWS)
    g = g.transpose(0, 3, 2, 1)  # [g, r, c, p]
    return np.ascontiguousarray(g.reshape(n_groups * GROUP_ROWS, DIM))


def _patch_tile_drain():
    """Workaround: this walrus build cannot encode semaphore waits on a
    sequencer Drain/NoOp with >1 wait ("Too many sync wait commands").
    Re-emit the TileContext tail waits as one nop per semaphore."""
    from concourse import mybir, tile
    from concourse.vector_clock import ScopedClock

    if getattr(tile.TileContext, "_drain_patched", False):
        return

    def _drain_and_barrier(self, tick_clock, wait_clock):
        nop_inst = self.nc.sync.nop(nofuse=True)
        wait_clock.add_sem_waits(
            nop_inst.ins, ScopedClock({None: tick_clock.global_clock})
        )
        si = nop_inst.ins.sync_info
        if si is not None and si.on_wait and len(si.on_wait) > 1:
            extra = si.on_wait[1:]
            si.on_wait = si.on_wait[:1]
            for w in extra:
                extra_nop = self.nc.sync.nop(nofuse=True)
                esi = extra_nop.ins.sync_info
                if esi is None:
                    extra_nop.ins.sync_info = mybir.SyncInfo(on_wait=[w], on_update=[])
                else:
                    esi.on_wait = list(esi.on_wait or []) + [w]
        self.nc.sync.drain()
        self.nc.all_engine_barrier()
        assert self.sems is not None
        popped = self.nc._tile_sem_poison_stack.pop()
        assert popped is self._sem_poison
        self.nc.clear_and_free_semaphores(list(self.sems.allocated().values()))
        self.nc.all_engine_barrier()

    tile.TileContext._drain_and_barrier = _drain_and_barrier
    tile.TileContext._drain_patched = True


def _split_multi_waits(nc, limit=1):
    """This walrus build encodes at most `limit` semaphore wait(s) per
    instruction ("Too many sync wait commands").  Hoist excess waits onto
    same-engine NoOps inserted immediately before the instruction."""
    from concourse import mybir

    counter = [0]

    def fresh_nop(engine, waits):
        counter[0] += 1
        nop = mybir.InstNoOp(
            name=f"waitsplit-{counter[0]}",
            engine=engine,
            ins=[],
            outs=[],
            bass_nofuse=True,
            sync_info=mybir.SyncInfo(on_wait=list(waits), on_update=[]),
        )
        nc.register_instruction(nop, overwrite=True)
        return nop

    for fn in nc.m.functions:
        for bb in fn.blocks:
            changed = False
            new = []
            for inst in bb.instructions:
                si = getattr(inst, "sync_info", None)
                if si is not None and si.on_wait and len(si.on_wait) > limit:
                    extra = si.on_wait[: len(si.on_wait) - limit]
                    si.on_wait = si.on_wait[len(si.on_wait) - limit :]
                    for k in range(0, len(extra), limit):
                        new.append(fresh_nop(inst.engine, extra[k : k + limit]))
                    changed = True
                new.append(inst)
            if changed:
                bb.instructions = new
    return nc


def build_bass(n_groups=N_GROUPS, reps=1, upto="full", load_split=4,
               store_rings="swdge", pair_psum=True, store_grain=2048):
    """Build the Bass module for one core processing n_groups row-groups.
    reps>1 repeats the whole pipeline in-NEFF (for timing calibration).
    upto: 'pe' | 'evict' | 'dve' | 'full' truncates the pipeline (for
    engine-attribution benchmarks).  load_split: number of load DMAs per
    group.  store_rings: 'act' | 'split' (alternate SP/ACT).  pair_psum:
    evict stage-8/9 chunk pairs as one [128,2048] ACT op."""
    _patch_tile_drain()
    from concourse import bass, mybir, tile

    stage_n = ["pe", "evict", "dve", "full"].index(upto)
    f16 = mybir.dt.float16
    f32 = mybir.dt.float32
    nc = bass.Bass("TRN2", target_bir_lowering=False, debug=False)
    xin = nc.dram_tensor("xin", [n_groups, 128, 8192], f16, kind="ExternalInput")
    wbd = nc.dram_tensor("wb", [128, 24, 128], f16, kind="ExternalInput")
    trig = nc.dram_tensor("trig", [128, 12], f32, kind="ExternalInput")
    yout = nc.dram_tensor("yout", [n_groups, 128, 8192], f16, kind="ExternalOutput")

    mult = mybir.AluOpType.mult
    add = mybir.AluOpType.add

    # PE emission order: path-B stage-8 partners adjacent, then path-A in
    # the order the stage-9 waves consume them.
    PE_ORDER = [0, 2, 1, 3, 4, 6, 5, 7]

    def mm_chunk(psum, wtile, c, xt):
        """Accumulate output chunk c's [128, 1024] PSUM tile."""
        if c < 4:
            idxs = [c * 2 + t for t in range(2)]
            cis = [2 * (c // 2) + t for t in range(2)]
        else:
            idxs = [8 + (c - 4) * 4 + t for t in range(4)]
            cis = [4 + t for t in range(4)]
        for h in range(2):
            for j, (i, ci) in enumerate(zip(idxs, cis)):
                nc.tensor.matmul(
                    psum[:, h * 512 : (h + 1) * 512],
                    wtile[:, i, :],
                    xt[:, ci * 1024 + h * 512 : ci * 1024 + (h + 1) * 512],
                    start=(j == 0),
                    stop=(j == len(idxs) - 1),
                )

    with tile.TileContext(nc) as tc:
        with (
            tc.tile_pool(name="wp", bufs=1) as wp,
            tc.tile_pool(name="xp", bufs=3) as xp,
            tc.tile_pool(name="yp", bufs=2) as yp,
            tc.tile_pool(name="ep", bufs=12) as ep,
            tc.tile_pool(name="qp", bufs=6) as qp,
            tc.tile_pool(name="ps", bufs=2 if pair_psum else 4, space="PSUM") as psp,
        ):
            wb = wp.tile([128, 24, 128], f16)
            nc.sync.dma_start(wb[:], wbd.ap()[:])
            tg = wp.tile([128, 12], f32)
            nc.sync.dma_start(tg[:], trig.ap()[:])

            for g in [g for _ in range(reps) for g in range(n_groups)]:
                xt = xp.tile([128, 8192], f16)
                lw = 8192 // load_split
                for ls in range(load_split):
                    nc.sync.dma_start(
                        xt[:, ls * lw : (ls + 1) * lw],
                        xin.ap()[g][:, ls * lw : (ls + 1) * lw],
                    )
                yt = yp.tile([128, 8192], f16)

                # PE + ACT eviction per chunk (or per stage-8/9 pair)
                w = {}
                if pair_psum:
                    for ca, cb in ((0, 2), (1, 3), (4, 6), (5, 7)):
                        p = psp.tile([128, 2048], f32, tag="ps")
                        mm_chunk(p[:, 0:1024], wb, ca, xt)
                        mm_chunk(p[:, 1024:2048], wb, cb, xt)
                        if stage_n >= 1:
                            e = ep.tile(
                                [128, 2048], f16, tag="e", name=f"w{ca}{cb}"
                            )
                            nc.scalar.copy(e[:], p[:])
                            w[ca], w[cb] = e[:, 0:1024], e[:, 1024:2048]
                else:
                    for c in PE_ORDER:
                        p = psp.tile([128, 1024], f32, tag="ps")
                        mm_chunk(p, wb, c, xt)
                        if stage_n >= 1:
                            e = ep.tile([128, 1024], f16, tag="e", name=f"w{c}")
                            nc.scalar.copy(e[:], p[:])
                            w[c] = e[:]
                if stage_n < 2:
                    continue

                # DVE stage 8 (path-B pairs): q'A = coefA*wB + wA, etc.
                q = {}
                for j, (A, B, _) in enumerate(S8_PAIRS):
                    qA = qp.tile([128, 1024], f16, tag="q", name=f"q{A}")
                    nc.vector.scalar_tensor_tensor(
                        qA[:], w[B], tg[:, 8 + 2 * j : 9 + 2 * j], w[A],
                        mult, add,
                    )
                    qB = qp.tile([128, 1024], f16, tag="q", name=f"q{B}")
                    nc.vector.scalar_tensor_tensor(
                        qB[:], w[A], tg[:, 9 + 2 * j : 10 + 2 * j], w[B],
                        mult, add,
                    )
                    q[A], q[B] = qA[:], qB[:]
                for c in (4, 5, 6, 7):
                    q[c] = w[c]

                # DVE stage 9: pairs (cg, cg+4) into the output tile.  All
                # lo outputs (slots 0..3) first so their stores fire while
                # the hi waves still run.
                def store(sl):
                    # stores ride SWDGE (Pool engine, otherwise idle): the
                    # issuing engine pays ~1.5-2 ns/KB of descriptor-gen, so
                    # putting stores on ACT (which also evicts) serializes
                    # against the evictions
                    eng = {"act": nc.scalar, "swdge": nc.gpsimd}[store_rings]
                    eng.dma_start(
                        yout.ap()[g][:, sl : sl + store_grain],
                        yt[:, sl : sl + store_grain],
                    )

                for cg in range(4):
                    nc.vector.scalar_tensor_tensor(
                        yt[:, cg * 1024 : (cg + 1) * 1024],
                        q[cg + 4], tg[:, 4 + cg : 5 + cg], q[cg],
                        mult, add,
                    )
                    if stage_n >= 3 and (cg + 1) * 1024 % store_grain == 0:
                        store((cg + 1) * 1024 - store_grain)
                for cg in range(4):
                    nc.vector.scalar_tensor_tensor(
                        yt[:, (cg + 4) * 1024 : (cg + 5) * 1024],
                        q[cg], tg[:, cg : cg + 1], q[cg + 4],
                        mult, add,
                    )
                    if stage_n >= 3 and (cg + 1) * 1024 % store_grain == 0:
                        store((cg + 5) * 1024 - store_grain)
    _split_multi_waits(nc)
    return nc


_CACHE = {}


def _get_nc(n_groups=N_GROUPS):
    if n_groups not in _CACHE:
        _CACHE[n_groups] = build_bass(n_groups)
    return _CACHE[n_groups]


def make_in_maps(x, angles):
    """Pack full inputs into per-core in_maps (list of dicts)."""
    x = np.asarray(x, dtype=np.float32)
    angles = np.asarray(angles, dtype=np.float32)
    wb, trig = _host_tables(angles)
    flat = x.reshape(-1, DIM).astype(np.float16)
    in_maps = []
    for k in range(N_CORES):
        shard = flat[k * ROWS_PER_CORE : (k + 1) * ROWS_PER_CORE]
        in_maps.append({"xin": _pack_x(shard), "wb": wb, "trig": trig})
    return in_maps


def kernel(x, angles):
    from concourse.bass_utils import run_bass_kernel_spmd

    x = np.asarray(x)
    orig_shape = x.shape
    in_maps = make_in_maps(x, angles)
    nc = _get_nc()
    res = run_bass_kernel_spmd(nc, in_maps, core_ids=list(range(N_CORES)))
    parts = [_unpack_y(res.results[k]["yout"]) for k in range(N_CORES)]
    out = np.concatenate(parts, axis=0).reshape(orig_shape)
    return out.astype(np.float32)
